# revision 8
# baseline (speedup 1.0000x reference)
"""DGCNN GNN layers on 8 Trainium2 NeuronCores (Bass/Tile).

Strategy (data-parallel over graphs, 64 graphs/core):
  - Host: filter intra-graph edges, compute deg/rinv, bake scaling into edge
    weights (w'' = w * rinv[ls] * deg[ld]), bucket edges per graph (K slots).
  - Device per graph: build scaled-transposed adjacency AT = S''^T + I densely
    in SBUF via one-hot outer-product matmuls (E1^T @ E2), then run the 4
    DGCNN layers as matmuls:
        z   = h @ W            (block-diag W over 4 stacked graphs)
        zh  = rinv * z         (DVE broadcast multiply)
        y^T = zh^T @ AT        (col-group packed matmuls, 4 graphs/PSUM tile)
        h'  = tanh(y^T + b)    (one ACT instr per 4 graphs)
  - Adjacency never touches HBM; x is streamed once.
"""

import numpy as np
import concourse.bass as bass
import concourse.bacc as bacc
import concourse.mybir as mybir
from concourse import tile
from concourse.bass_utils import run_bass_kernel_spmd

B, P, F, E = 512, 256, 64, 2 * 1024 * 1024
NCORES = 8
G = B // NCORES            # 64 graphs per core
DEF_K = 64                 # edge slots per graph (Poisson(8) tail-safe)
FOUT = [32, 32, 32, 32]    # layer-3 padded from 1 to 32
Alu = mybir.AluOpType
Act = mybir.ActivationFunctionType
DT = mybir.dt.float32

_prog_cache = {}


def _build_program(K):
    R = 128 // K               # graphs stacked per edge/build tile
    npairs = G // R
    nc = bacc.Bacc()
    xT_d = nc.declare_dram_parameter("xT", [G, F, P], DT, isOutput=False)
    ed_d = nc.declare_dram_parameter("edat", [R * K, npairs, 3], DT, isOutput=False)
    rc_d = nc.declare_dram_parameter("rc", [128, 2 * G], DT, isOutput=False)
    iota_d = nc.declare_dram_parameter("iota", [128, P], DT, isOutput=False)
    eye_d = nc.declare_dram_parameter("eyewide", [128, 2 * P], DT, isOutput=False)
    wb_d = [nc.declare_dram_parameter(f"wblk{l}", [128, 64 if l == 0 else 128], DT,
                                      isOutput=False) for l in range(4)]
    bs_d = [nc.declare_dram_parameter(f"bstk{l}", [128, 1], DT, isOutput=False)
            for l in range(4)]
    out_d = nc.declare_dram_parameter("out", [G, P], DT, isOutput=True)

    with tile.TileContext(nc) as tc:
        with (
            tc.tile_pool(name="const", bufs=1) as cpool,
            tc.tile_pool(name="epool", bufs=3) as epool,
            tc.tile_pool(name="atpool", bufs=16) as atpool,
            tc.tile_pool(name="hpool", bufs=4) as hpool,
            tc.tile_pool(name="zhpool", bufs=3) as zhpool,
            tc.tile_pool(name="opool", bufs=2) as opool,
            tc.tile_pool(name="ps_at", bufs=3, space="PSUM") as ps_at,
            tc.tile_pool(name="ps_z", bufs=2, space="PSUM") as ps_z,
            tc.tile_pool(name="ps_y", bufs=2, space="PSUM") as ps_y,
        ):
            iota = cpool.tile([128, P], DT)
            eye = cpool.tile([128, 2 * P], DT)
            rc = cpool.tile([128, 2 * G], DT)
            ed = cpool.tile([R * K, npairs, 3], DT)
            nc.sync.dma_start(iota[:], iota_d[:])
            nc.sync.dma_start(eye[:], eye_d[:])
            nc.sync.dma_start(rc[:], rc_d[:])
            nc.sync.dma_start(ed[:], ed_d[:])
            wblk, bstk = [], []
            for l in range(4):
                w = cpool.tile([128, 64 if l == 0 else 128], DT, tag=f"wb{l}")
                b = cpool.tile([128, 1], DT, tag=f"bs{l}")
                nc.sync.dma_start(w[:], wb_d[l][:])
                nc.sync.dma_start(b[:], bs_d[l][:])
                wblk.append(w)
                bstk.append(b)

            for q in range(G // 4):            # quad of graphs 4q..4q+3
                h_a = hpool.tile([128, P], DT, tag="h0a")
                h_b = hpool.tile([128, P], DT, tag="h0b")
                for j in range(4):
                    tgt, r = (h_a, j) if j < 2 else (h_b, j - 2)
                    nc.sync.dma_start(tgt[64 * r:64 * (r + 1), :], xT_d[4 * q + j])

                # ---- adjacency build: AT[j][h] = S''^T half + I ----
                ats = [None] * 8
                for jj in range(4):
                    pnum = (4 * q + jj) // R
                    rr = (4 * q + jj) % R
                    if rr == 0:
                        E1 = epool.tile([R * K, P], DT, tag="E1")
                        E2 = epool.tile([R * K, P], DT, tag="E2")
                        nc.vector.tensor_tensor(
                            E1[:], iota[:R * K, :],
                            ed[:, pnum, 1:2].broadcast_to([R * K, P]), Alu.is_equal)
                        nc.vector.tensor_tensor(
                            E2[:], iota[:R * K, :],
                            ed[:, pnum, 0:1].broadcast_to([R * K, P]), Alu.is_equal)
                        nc.vector.tensor_tensor(
                            E2[:], E2[:],
                            ed[:, pnum, 2:3].broadcast_to([R * K, P]), Alu.mult)
                    for h in range(2):
                        pat = ps_at.tile([128, P], DT, tag="pat")
                        nc.tensor.matmul(
                            pat[:], E1[K * rr:K * (rr + 1), 128 * h:128 * (h + 1)],
                            E2[K * rr:K * (rr + 1), :], start=True, stop=True)
                        at = atpool.tile([128, P], DT, tag="at")
                        nc.vector.tensor_tensor(
                            at[:], pat[:], eye[:, P * h:P * (h + 1)], Alu.add)
                        ats[2 * jj + h] = at

                # ---- 4 layers ----
                hcur = (h_a, h_b)
                for l in range(4):
                    pz = ps_z.tile([128, P], DT, tag="pz")
                    if l == 0:
                        for t in range(2):
                            for h in range(2):
                                nc.tensor.matmul(
                                    pz[:, 128 * h + 64 * t:128 * h + 64 * (t + 1)],
                                    hcur[t][:, 128 * h:128 * (h + 1)],
                                    wblk[0][:], start=True, stop=True)
                    else:
                        for h in range(2):
                            nc.tensor.matmul(
                                pz[:, 128 * h:128 * (h + 1)],
                                hcur[:, 128 * h:128 * (h + 1)],
                                wblk[l][:], start=True, stop=True)
                    zh = zhpool.tile([128, P], DT, tag="zh")
                    rc_ap = (rc[:].rearrange("p (h g) -> p h g", h=2)
                             [:, :, 4 * q:4 * q + 4].broadcast_to([128, 2, 4, 32]))
                    nc.vector.tensor_tensor(
                        zh[:].rearrange("p (h j f) -> p h j f", h=2, j=4),
                        pz[:].rearrange("p (h j f) -> p h j f", h=2, j=4),
                        rc_ap, Alu.mult)
                    py = ps_y.tile([128, P], DT, tag="py")
                    for j in range(4):
                        for h in range(2):
                            nc.tensor.matmul(
                                py[32 * j:32 * (j + 1), :],
                                zh[:, 128 * h + 32 * j:128 * h + 32 * (j + 1)],
                                ats[2 * j + h][:],
                                start=(h == 0), stop=(h == 1),
                                tile_position=(0, 32 * j))
                    if l < 3:
                        hn = hpool.tile([128, P], DT, tag="hn")
                        nc.scalar.activation(hn[:], py[:], Act.Tanh, bias=bstk[l][:, 0:1])
                        hcur = hn
                    else:
                        ob = opool.tile([128, P], DT, tag="ob")
                        nc.scalar.activation(ob[:], py[:], Act.Tanh, bias=bstk[l][:, 0:1])
                        nc.sync.dma_start(
                            out_d[4 * q:4 * q + 4, :],
                            ob[:].rearrange("(j r) n -> j r n", r=32)[:, 0, :])
    nc.compile()
    nc.finalize()
    return nc


def _prep(edge_index, edge_mask):
    src = edge_index[0].astype(np.int64)
    dst = edge_index[1].astype(np.int64)
    valid = (src >> 8) == (dst >> 8)
    vs, vd, w = src[valid], dst[valid], edge_mask[valid].astype(np.float64)
    g, ls, ld = vs >> 8, vs & 255, vd & 255

    deg = np.ones((B, P), np.float64)
    np.add.at(deg, (g, ls), w)
    deg = deg.astype(np.float32)
    rinv = (1.0 / deg).astype(np.float32)
    w2 = (w.astype(np.float32) * rinv[g, ls] * deg[g, ld]).astype(np.float32)

    # bucket edges per graph
    order = np.argsort(g, kind="stable")
    g_s, ls_s, ld_s, w_s = g[order], ls[order], ld[order], w2[order]
    starts = np.searchsorted(g_s, np.arange(B))
    slot = np.arange(len(g_s)) - starts[g_s]
    K = DEF_K
    maxc = int(slot.max()) + 1 if len(slot) else 0
    if maxc > K:
        # merge duplicate (g, ls, ld) before giving up on K=64
        key = (g_s * 65536 + ls_s * 256 + ld_s)
        uk, inv = np.unique(key, return_inverse=True)
        wm = np.zeros(len(uk), np.float32)
        np.add.at(wm, inv, w_s)
        g_s, ls_s, ld_s, w_s = uk >> 16, (uk >> 8) & 255, uk & 255, wm
        starts = np.searchsorted(g_s, np.arange(B))
        slot = np.arange(len(g_s)) - starts[g_s]
        maxc = int(slot.max()) + 1 if len(slot) else 0
        while maxc > K:
            K *= 2
        assert K <= 128, f"too many edges per graph: {maxc}"

    LS = np.zeros((B, K), np.float32)
    LD = np.zeros((B, K), np.float32)
    WW = np.zeros((B, K), np.float32)
    LS[g_s, slot] = ls_s
    LD[g_s, slot] = ld_s
    WW[g_s, slot] = w_s
    return LS, LD, WW, rinv, K


def kernel(x, edge_index, batch, edge_mask, W0, b0, W1, b1, W2, b2, W3, b3):
    x = np.asarray(x, np.float32)
    LS, LD, WW, rinv, K = _prep(np.asarray(edge_index),
                                np.asarray(edge_mask, np.float32))

    R = 128 // K
    npairs = G // R

    # constants (shared across cores)
    iota = np.tile(np.arange(P, dtype=np.float32), (128, 1))
    eyewide = np.zeros((128, 2 * P), np.float32)
    eyewide[:, :128] = np.eye(128, dtype=np.float32)
    eyewide[np.arange(128), P + 128 + np.arange(128)] = 1.0
    Ws = [np.asarray(W0, np.float32), np.asarray(W1, np.float32),
          np.asarray(W2, np.float32), np.asarray(W3, np.float32)]
    bs = [np.asarray(b0, np.float32), np.asarray(b1, np.float32),
          np.asarray(b2, np.float32), np.asarray(b3, np.float32)]
    W3p = np.zeros((32, 32), np.float32)
    W3p[:, :1] = Ws[3]
    b3p = np.zeros(32, np.float32)
    b3p[0] = bs[3][0]
    wblks = []
    wblks.append(np.block([[Ws[0], np.zeros((64, 32), np.float32)],
                           [np.zeros((64, 32), np.float32), Ws[0]]]))
    for l in (1, 2):
        z = np.zeros((128, 128), np.float32)
        for j in range(4):
            z[32 * j:32 * (j + 1), 32 * j:32 * (j + 1)] = Ws[l]
        wblks.append(z)
    z = np.zeros((128, 128), np.float32)
    for j in range(4):
        z[32 * j:32 * (j + 1), 32 * j:32 * (j + 1)] = W3p
    wblks.append(z)
    bstks = [np.tile(bs[0], 4)[:, None], np.tile(bs[1], 4)[:, None],
             np.tile(bs[2], 4)[:, None], np.tile(b3p, 4)[:, None]]

    key = K
    if key not in _prog_cache:
        _prog_cache[key] = _build_program(K)
    nc = _prog_cache[key]

    xg = np.ascontiguousarray(
        x.reshape(B, P, F).transpose(0, 2, 1))         # [B, F, P]
    in_maps = []
    for c in range(NCORES):
        sl = slice(c * G, (c + 1) * G)
        edat = np.stack([
            LS[sl].reshape(npairs, R, K).transpose(1, 2, 0).reshape(R * K, npairs),
            LD[sl].reshape(npairs, R, K).transpose(1, 2, 0).reshape(R * K, npairs),
            WW[sl].reshape(npairs, R, K).transpose(1, 2, 0).reshape(R * K, npairs),
        ], axis=-1)                                     # [R*K, npairs, 3]
        rcc = np.ascontiguousarray(
            rinv[sl].reshape(G, 2, 128).transpose(2, 1, 0).reshape(128, 2 * G))
        m = {"xT": np.ascontiguousarray(xg[sl]),
             "edat": np.ascontiguousarray(edat), "rc": rcc,
             "iota": iota, "eyewide": eyewide}
        for l in range(4):
            m[f"wblk{l}"] = np.ascontiguousarray(wblks[l])
            m[f"bstk{l}"] = np.ascontiguousarray(bstks[l])
        in_maps.append(m)

    res = run_bass_kernel_spmd(nc, in_maps, list(range(NCORES)))
    out = np.concatenate([res.results[c]["out"] for c in range(NCORES)], axis=0)
    return out.reshape(B, P, 1).astype(np.float32)
